# revision 30
# baseline (speedup 1.0000x reference)
"""Cross-attention kernel for 8 Trainium2 NeuronCores.

Sharding: 8 cores = 4 batches x 2 head-groups (6 heads each).
Per core (b, hg), with activations pre-tiled on host:
  qT = (Wq_hg*scale).T' @ xqT   [384, 2048]   (weights column-split)
  kT = Wk_hg' @ xkT             [384, 2048]
  v  = xvT' @ Wv_hg.T           [2048, 384]  (+ ones column per head)
  per head h: lt = k_h qT_h     [2048k, 2048q] (logits transposed)
              p  = exp(lt)      (no max-subtraction: logits are O(1))
              [x; d] = [v_h|1].T @ p   -> x rows 0..63, denominators row 64
              xn = x * (1/d)    (gpsimd partition-broadcast of 1/d)
  outT_partial = Wo_hg.T' @ xn  [768, 2048]
Host: out[b] = (partial[2b] + partial[2b+1]).T + bo.

The engine queues are in-order, so emission order is a static schedule:
inputs stream in query/key column-chunks (host pre-tiled so every DMA is
per-partition contiguous - 128 descriptors); q0/k0 projections chase the
chunk DMAs so the first exp issues ~10us in; the v/q/k/out projections
interleave into the attention stream as PE filler; softmax normalization
routes the denominator row to partition 0 via a small SBUF DMA (DVE and
gpsimd ops mis-handle non-zero partition bases on HW), then
reciprocal_approx_fast + gpsimd partition_broadcast.  The last unit is
processed in half-chunks to overlap the output-projection tail.
All matmuls bf16 inputs with fp32 PSUM accumulation.
"""

import sys

import numpy as np

for _p in ("/opt/trn_rl_repo",):
    if _p not in sys.path:
        sys.path.insert(0, _p)

B, NQ, NK, C = 4, 2048, 2048, 768
H, DH = 12, 64
HPC, HB = 6, 384  # heads per core, head-block width
P = 128
KT = C // P  # 6 contraction k-tiles for projections
QCH = 512  # query-chunk width
NCH = NQ // QCH  # 4 query chunks
NKT = NK // P  # 16 key tiles
SCALE = DH**-0.5  # folded into the exp activation scale (exactly 0.125)
VW = DH + 1  # v block width per head incl. ones column

_prog = None


def _build():
    from contextlib import ExitStack

    import concourse.bass as bass
    import concourse.tile as tile
    from concourse import library_config, mybir
    from concourse.bacc import Bacc

    f32 = mybir.dt.float32
    bf16 = mybir.dt.bfloat16
    fp8 = mybir.dt.float8e4
    EXP = mybir.ActivationFunctionType.Exp
    DR = mybir.MatmulPerfMode.DoubleRow

    nc = Bacc()
    xq_d = nc.declare_dram_parameter("xq", [NCH, P, KT, QCH], bf16, isOutput=False)
    xk_d = nc.declare_dram_parameter("xk", [NCH, P, KT, QCH], bf16, isOutput=False)
    xv_d = nc.declare_dram_parameter("xv", [P, KT, NK], bf16, isOutput=False)
    wq_d = nc.declare_dram_parameter("wq", [P, KT, HB], bf16, isOutput=False)
    wk_d = nc.declare_dram_parameter("wk", [P, KT, HB], bf16, isOutput=False)
    wv_d = nc.declare_dram_parameter("wv", [P, KT, HB], bf16, isOutput=False)
    wo_d = nc.declare_dram_parameter("wo", [P, HB // P, C], bf16, isOutput=False)
    out_d = nc.declare_dram_parameter("out", [C, NQ], f32, isOutput=True)

    with tile.TileContext(nc) as tc, ExitStack() as ctx:
        const = ctx.enter_context(tc.tile_pool(name="const", bufs=1))
        xin = ctx.enter_context(tc.tile_pool(name="xin", bufs=NCH))
        qk = ctx.enter_context(tc.tile_pool(name="qk", bufs=3))
        pp = ctx.enter_context(tc.tile_pool(name="pp", bufs=21))
        xnp = ctx.enter_context(tc.tile_pool(name="xnp", bufs=3))
        small = ctx.enter_context(tc.tile_pool(name="small", bufs=2))
        ost = ctx.enter_context(tc.tile_pool(name="ost", bufs=3))
        proj_ps = ctx.enter_context(tc.tile_pool(name="proj_ps", bufs=2, space="PSUM"))
        lt_ps = ctx.enter_context(tc.tile_pool(name="lt_ps", bufs=2, space="PSUM"))
        x_ps = ctx.enter_context(tc.tile_pool(name="x_ps", bufs=2, space="PSUM"))

        nc.gpsimd.load_library(library_config.attn)

        # pre-warm the Exp activation table while DMAs stream
        pw_in = const.tile([1, 16], f32, tag="pw_in")
        pw_out = const.tile([1, 16], f32, tag="pw_out")
        nc.vector.memset(pw_in, 0.0)
        nc.scalar.activation(pw_out, pw_in, EXP)
        # warm the PE p-state ramp with throwaway matmuls while DMAs stream
        wu = const.tile([P, P], bf16, tag="wu")
        nc.vector.memset(wu, 0.0)

        # ---- weights + inputs to SBUF.  The sync DMA queue drains in
        # emission order; this order paces the whole schedule.
        wq_s = const.tile([P, KT, HB], bf16, tag="wq")
        wk_s = const.tile([P, KT, HB], bf16, tag="wk")
        wv_s = const.tile([P, KT, HB], bf16, tag="wv")
        wo_s = const.tile([P, HB // P, C], bf16, tag="wo")
        xv_s = const.tile([P, KT, NK], bf16, tag="xv")

        def csl(c):
            return slice(c * QCH, (c + 1) * QCH)

        xq_c = [xin.tile([P, KT, QCH], bf16, tag="xq", name=f"xq{c}") for c in range(NCH)]
        xk_c = [xin.tile([P, KT, QCH], bf16, tag="xk", name=f"xk{c}") for c in range(NCH)]

        # only the mt=0 weight slices gate the first projections
        nc.sync.dma_start(out=xq_c[0], in_=xq_d[0, :, :, :])
        nc.sync.dma_start(out=wq_s[:, :, 0:P], in_=wq_d[:, :, 0:P])
        nc.sync.dma_start(out=wk_s[:, :, 0:P], in_=wk_d[:, :, 0:P])
        nc.sync.dma_start(out=xk_c[0], in_=xk_d[0, :, :, :])
        nc.sync.dma_start(out=wv_s, in_=wv_d[:, :, :])
        nc.sync.dma_start(out=xv_s[:, :, 0 : NK // 2], in_=xv_d[:, :, 0 : NK // 2])
        nc.sync.dma_start(out=xk_c[1], in_=xk_d[1, :, :, :])
        nc.sync.dma_start(out=xv_s[:, :, NK // 2 : NK], in_=xv_d[:, :, NK // 2 : NK])
        nc.sync.dma_start(out=xk_c[2], in_=xk_d[2, :, :, :])
        nc.sync.dma_start(out=xk_c[3], in_=xk_d[3, :, :, :])
        nc.sync.dma_start(out=xq_c[1], in_=xq_d[1, :, :, :])
        nc.sync.dma_start(out=wq_s[:, :, P:HB], in_=wq_d[:, :, P:HB])
        nc.sync.dma_start(out=wk_s[:, :, P:HB], in_=wk_d[:, :, P:HB])
        nc.sync.dma_start(out=xq_c[2], in_=xq_d[2, :, :, :])
        nc.sync.dma_start(out=xq_c[3], in_=xq_d[3, :, :, :])
        nc.sync.dma_start(out=wo_s, in_=wo_d[:, :, :])

        # v with a ones column per head: [128, kt, head, 65]
        v_s = const.tile([P, NKT, HPC, VW], bf16, tag="v")
        nc.vector.memset(v_s[:, :, :, DH : DH + 1], 1.0)

        qT_t = [qk.tile([P, NQ], bf16, tag="qT", name=f"qT{i}") for i in range(3)]
        kT_t = [qk.tile([P, NK], bf16, tag="kT", name=f"kT{i}") for i in range(3)]
        xn_t = [xnp.tile([P, NQ], bf16, tag="xn", name=f"xn{i}") for i in range(3)]

        def proj_chunk(w_s, src_c, dst, mt, c):
            ps = proj_ps.tile([P, QCH], f32, tag="proj")
            for k in range(KT):
                nc.tensor.matmul(
                    ps,
                    w_s[:, k, mt * P : (mt + 1) * P],
                    src_c[c][:, k, :],
                    start=(k == 0),
                    stop=(k == KT - 1),
                )
            nc.vector.tensor_copy(dst[:, csl(c)], ps)

        def v_proj_kt(kt):
            ps = proj_ps.tile([P, QCH], f32, tag="proj")
            psv = ps[:, 0:HB]
            for k in range(KT):
                nc.tensor.matmul(
                    psv,
                    xv_s[:, k, kt * P : (kt + 1) * P],
                    wv_s[:, k, :],
                    start=(k == 0),
                    stop=(k == KT - 1),
                )
            nc.vector.tensor_copy(
                v_s[:, kt, :, 0:DH], psv.rearrange("p (h m) -> p h m", m=DH)
            )

        rows = (slice(0, DH), slice(DH, 2 * DH))
        p_store = {}  # (pair, chunk) -> [hh][ktp] p tiles

        def qk_ktp(p3, c, ktp):
            # logits for key tiles (2*ktp, 2*ktp+1), both heads of the pair
            lts = [
                lt_ps.tile([P, 2 * QCH], f32, tag="lt", name=f"lt{i}") for i in range(2)
            ]
            for u in range(2):
                kt = 2 * ktp + u
                for hh in range(2):
                    nc.tensor.matmul(
                        lts[hh][:, u * QCH : (u + 1) * QCH],
                        kT_t[p3][rows[hh], kt * P : (kt + 1) * P],
                        qT_t[p3][rows[hh], csl(c)],
                        start=True,
                        stop=True,
                    )
            for hh in range(2):
                pt = pp.tile([P, 2 * QCH], bf16, tag="p")
                # attention scale folded into the exp: exp(logit/8)
                nc.scalar.activation(pt, lts[hh], EXP, scale=SCALE)
                p_store[(p3, c)][hh].append(pt)

        def qk_unit(p3, c, ktps=range(NKT // 2)):
            if (p3, c) not in p_store:
                p_store[(p3, c)] = ([], [])
            for ktp in ktps:
                qk_ktp(p3, c, ktp)

        def av_norm(p3, c, hh, ptiles, q0, qw):
            # AV chain + normalize for query columns [q0, q0+qw) of chunk c
            h = 2 * p3 + hh
            xps = x_ps.tile([DH + 1, QCH], f32, tag="x")
            xpsw = xps[:, 0:qw]
            for kt in range(NKT):
                pt = ptiles[hh][kt // 2][:, (kt % 2) * QCH + q0 :][:, 0:qw]
                nc.tensor.matmul(
                    xpsw,
                    v_s[:, kt, h, :],
                    pt,
                    start=(kt == 0),
                    stop=(kt == NKT - 1),
                )
            qsl = slice(c * QCH + q0, c * QCH + q0 + qw)
            # copy x+denom to SBUF (frees the PSUM bank); the custom-DVE
            # recip and gpsimd broadcast mis-handle non-zero partition bases
            # on HW, so DMA the denom row to a partition-0 tile first.
            xsb = small.tile([DH + 1, QCH], f32, tag="xsb", bufs=3)
            nc.vector.tensor_copy(xsb[:, 0:qw], xpsw)
            r0 = small.tile([1, QCH], f32, tag="r0")
            nc.sync.dma_start(out=r0[:, 0:qw], in_=xsb[DH : DH + 1, 0:qw])
            r = small.tile([1, QCH], f32, tag="r")
            nc.vector.reciprocal_approx_fast(r[:, 0:qw], r0[:, 0:qw])
            rb = small.tile([DH, QCH], f32, tag="rb", bufs=2)
            nc.gpsimd.partition_broadcast(rb[:, 0:qw], r[:, 0:qw])
            if hh == 0:
                nc.vector.tensor_mul(xn_t[p3][0:DH, qsl], xsb[0:DH, 0:qw], rb[:, 0:qw])
            else:
                # DVE lanes are partition-locked; route rows 64..127
                # through a bounce tile + SBUF-to-SBUF DMA.
                tmp = small.tile([DH, QCH], bf16, tag="tmp")
                nc.vector.tensor_mul(tmp[:, 0:qw], xsb[0:DH, 0:qw], rb[:, 0:qw])
                nc.sync.dma_start(out=xn_t[p3][DH : 2 * DH, qsl], in_=tmp[:, 0:qw])

        def av_unit(p3, c):
            ptiles = p_store.pop((p3, c))
            for hh in range(2):
                av_norm(p3, c, hh, ptiles, 0, QCH)

        def outproj(c, q0=0, qw=QCH):
            qsl = slice(c * QCH + q0, c * QCH + q0 + qw)
            for mt in range(C // P):
                ps = proj_ps.tile([P, QCH], f32, tag="proj")
                psw = ps[:, 0:qw]
                for k3 in range(HB // P):
                    nc.tensor.matmul(
                        psw,
                        wo_s[:, k3, mt * P : (mt + 1) * P],
                        xn_t[k3][:, qsl],
                        start=(k3 == 0),
                        stop=(k3 == HB // P - 1),
                    )
                o = ost.tile([P, QCH], f32, tag="o")
                nc.vector.tensor_copy(o[:, 0:qw], psw)
                nc.sync.dma_start(
                    out=out_d[mt * P : (mt + 1) * P, qsl], in_=o[:, 0:qw]
                )

        # ---- static schedule.
        # Head: q0/k0 projections chase the chunked DMAs; QK ktp j only
        # needs key chunk j//2, so the exp stream starts ~10us in.
        for _ in range(30):
            wps = proj_ps.tile([P, QCH], f32, tag="proj")
            nc.tensor.matmul(wps[:, 0:P], wu, wu, start=True, stop=True)
        proj_chunk(wq_s, xq_c, qT_t[0], 0, 0)
        proj_chunk(wk_s, xk_c, kT_t[0], 0, 0)
        qk_unit(0, 0, ktps=(0, 1))
        proj_chunk(wk_s, xk_c, kT_t[0], 0, 1)
        qk_unit(0, 0, ktps=(2, 3))
        # v-projection rides inside the u0 QK window (which otherwise waits
        # on the chunked xk DMAs); AV(u0) halves land right where exp(u0)
        # drains so the p-tile pool frees for exp(u1) without a stall
        for kt in range(0, 4):
            v_proj_kt(kt)
        proj_chunk(wk_s, xk_c, kT_t[0], 0, 2)
        qk_unit(0, 0, ktps=(4, 5))
        for kt in range(4, 8):
            v_proj_kt(kt)
        proj_chunk(wk_s, xk_c, kT_t[0], 0, 3)
        qk_unit(0, 0, ktps=(6, 7))
        for kt in range(8, 12):
            v_proj_kt(kt)
        proj_chunk(wq_s, xq_c, qT_t[0], 0, 1)
        qk_unit(0, 1, ktps=(0, 1))
        for kt in range(12, 16):
            v_proj_kt(kt)
        pt_u0 = p_store.pop((0, 0))
        av_norm(0, 0, 0, pt_u0, 0, QCH)
        qk_ktp(0, 1, 2)
        av_norm(0, 0, 1, pt_u0, 0, QCH)
        for j in range(3, 8):
            qk_ktp(0, 1, j)

        units = [(p3, c) for p3 in range(3) for c in range(NCH)]
        # prerequisite projection chunks emitted just before QK(unit i);
        # the q/k chunks for pair p+1 are spread across pair p's units.
        fillers = {
            2: [("q", 0, 2), ("k", 1, 0)],
            3: [("q", 0, 3), ("k", 1, 1), ("k", 1, 2)],
            4: [("k", 1, 3), ("q", 1, 0)],
            5: [("q", 1, 1), ("k", 2, 0)],
            6: [("q", 1, 2), ("k", 2, 1)],
            7: [("q", 1, 3), ("k", 2, 2)],
            8: [("k", 2, 3), ("q", 2, 0)],
            9: [("q", 2, 1)],
            10: [("q", 2, 2)],
            11: [("q", 2, 3)],
        }

        def emit_filler(i):
            for f in fillers.get(i, ()):
                if f[0] == "q":
                    proj_chunk(wq_s, xq_c, qT_t[f[1]], f[1], f[2])
                else:
                    proj_chunk(wk_s, xk_c, kT_t[f[1]], f[1], f[2])

        pending_outproj = None
        for i, (p3, c) in enumerate(units):
            if i == 0:
                continue  # handled in the prologue above
            if i + 1 < len(units):
                emit_filler(i + 1)
                qk_unit(*units[i + 1])
            if pending_outproj is not None:
                outproj(pending_outproj)
                pending_outproj = None
            if i < len(units) - 1:
                av_unit(p3, c)
            else:
                # last unit: process in half-chunks so the final
                # output-projection overlaps the second half's AV/normalize
                ptiles = p_store.pop((p3, c))
                W1 = 3 * QCH // 4
                for hh in range(2):
                    av_norm(p3, c, hh, ptiles, 0, W1)
                for hh in range(2):
                    av_norm(p3, c, hh, ptiles, W1, QCH - W1)
                outproj(c, 0, W1)
                outproj(c, W1, QCH - W1)
            if p3 == 2:
                pending_outproj = c

    nc.finalize()
    return nc


def _get_prog():
    global _prog
    if _prog is None:
        _prog = _build()
    return _prog


def _shard_inputs(query, key, value, Wq, Wk, Wv, Wo):
    from ml_dtypes import bfloat16

    def qtile(x):  # [NQ, C] -> [NCH, P, KT, QCH], per-partition contiguous
        return np.ascontiguousarray(
            x.reshape(NCH, QCH, KT, P).transpose(0, 3, 2, 1)
        ).astype(bfloat16)

    def vtile(x):  # [NK, C] -> [P, KT, NK]
        return np.ascontiguousarray(x.T.reshape(KT, P, NK).transpose(1, 0, 2)).astype(
            bfloat16
        )

    def wtile(w):  # [C, HB] -> [P, KT, HB]
        return np.ascontiguousarray(w.reshape(KT, P, HB).transpose(1, 0, 2)).astype(
            bfloat16
        )

    in_maps = []
    for core in range(8):
        b, hg = core // 2, core % 2
        sl = slice(hg * HB, (hg + 1) * HB)
        wo_t = Wo[:, sl].T  # [HB, C]
        in_maps.append(
            {
                "xq": qtile(np.asarray(query[b])),
                "xk": qtile(np.asarray(key[b])),
                "xv": vtile(np.asarray(value[b])),
                "wq": wtile(np.ascontiguousarray(Wq[sl, :].T)),
                "wk": wtile(np.ascontiguousarray(Wk[sl, :].T)),
                "wv": wtile(np.ascontiguousarray(Wv[sl, :].T)),
                "wo": np.ascontiguousarray(
                    wo_t.reshape(HB // P, P, C).transpose(1, 0, 2)
                ).astype(bfloat16),
            }
        )
    return in_maps


def kernel(query, key, value, Wq, Wk, Wv, Wo, bo):
    query, key, value = np.asarray(query), np.asarray(key), np.asarray(value)
    Wq, Wk, Wv, Wo = np.asarray(Wq), np.asarray(Wk), np.asarray(Wv), np.asarray(Wo)
    bo = np.asarray(bo).astype(np.float32)

    from concourse.bass_utils import run_bass_kernel_spmd

    nc = _get_prog()
    in_maps = _shard_inputs(query, key, value, Wq, Wk, Wv, Wo)
    res = run_bass_kernel_spmd(nc, in_maps, list(range(8))).results

    out = np.empty((B, NQ, C), np.float32)
    for b in range(B):
        acc = res[2 * b]["out"].astype(np.float32) + res[2 * b + 1]["out"].astype(
            np.float32
        )
        out[b] = acc.T + bo[None, :]
    return out


# revision 31
# speedup vs baseline: 1.1827x; 1.1827x over previous
"""Cross-attention kernel for 8 Trainium2 NeuronCores.

Sharding: 8 cores = 4 batches x 2 head-groups (6 heads each).
Per core (b, hg), with activations pre-tiled on host:
  qT = (Wq_hg*scale).T' @ xqT   [384, 2048]   (weights column-split)
  kT = Wk_hg' @ xkT             [384, 2048]
  v  = xvT' @ Wv_hg.T           [2048, 384]  (+ ones column per head)
  per head h: lt = k_h qT_h     [2048k, 2048q] (logits transposed)
              p  = exp(lt)      (no max-subtraction: logits are O(1))
              [x; d] = [v_h|1].T @ p   -> x rows 0..63, denominators row 64
              xn = x * (1/d)    (gpsimd partition-broadcast of 1/d)
  outT_partial = Wo_hg.T' @ xn  [768, 2048]
Host: out[b] = (partial[2b] + partial[2b+1]).T + bo.

The engine queues are in-order, so emission order is a static schedule:
inputs stream in query/key column-chunks (host pre-tiled so every DMA is
per-partition contiguous - 128 descriptors); q0/k0 projections chase the
chunk DMAs so the first exp issues ~10us in; the v/q/k/out projections
interleave into the attention stream as PE filler; softmax normalization
routes the denominator row to partition 0 via a small SBUF DMA (DVE and
gpsimd ops mis-handle non-zero partition bases on HW), then
reciprocal_approx_fast + gpsimd partition_broadcast.  The last unit is
processed in half-chunks to overlap the output-projection tail.
All matmuls bf16 inputs with fp32 PSUM accumulation.
"""

import sys

import numpy as np

for _p in ("/opt/trn_rl_repo",):
    if _p not in sys.path:
        sys.path.insert(0, _p)

B, NQ, NK, C = 4, 2048, 2048, 768
H, DH = 12, 64
HPC, HB = 6, 384  # heads per core, head-block width
P = 128
KT = C // P  # 6 contraction k-tiles for projections
QCH = 512  # query-chunk width
NCH = NQ // QCH  # 4 query chunks
NKT = NK // P  # 16 key tiles
SCALE = DH**-0.5  # folded into the exp activation scale (exactly 0.125)
VW = DH + 1  # v block width per head incl. ones column

_prog = None


def _build():
    from contextlib import ExitStack

    import concourse.bass as bass
    import concourse.tile as tile
    from concourse import library_config, mybir
    from concourse.bacc import Bacc

    f32 = mybir.dt.float32
    bf16 = mybir.dt.bfloat16
    fp8 = mybir.dt.float8e4
    EXP = mybir.ActivationFunctionType.Exp
    DR = mybir.MatmulPerfMode.DoubleRow

    nc = Bacc()
    xq_d = nc.declare_dram_parameter("xq", [NCH, P, KT, QCH], bf16, isOutput=False)
    xk_d = nc.declare_dram_parameter("xk", [NCH, P, KT, QCH], bf16, isOutput=False)
    xv_d = nc.declare_dram_parameter("xv", [P, KT, NK], bf16, isOutput=False)
    wq_d = nc.declare_dram_parameter("wq", [P, KT, HB], bf16, isOutput=False)
    wk_d = nc.declare_dram_parameter("wk", [P, KT, HB], bf16, isOutput=False)
    wv_d = nc.declare_dram_parameter("wv", [P, KT, HB], bf16, isOutput=False)
    wo_d = nc.declare_dram_parameter("wo", [P, HB // P, C], bf16, isOutput=False)
    out_d = nc.declare_dram_parameter("out", [C, NQ], f32, isOutput=True)

    with tile.TileContext(nc) as tc, ExitStack() as ctx:
        const = ctx.enter_context(tc.tile_pool(name="const", bufs=1))
        xin = ctx.enter_context(tc.tile_pool(name="xin", bufs=NCH))
        qk = ctx.enter_context(tc.tile_pool(name="qk", bufs=3))
        pp = ctx.enter_context(tc.tile_pool(name="pp", bufs=21))
        xnp = ctx.enter_context(tc.tile_pool(name="xnp", bufs=3))
        small = ctx.enter_context(tc.tile_pool(name="small", bufs=2))
        ost = ctx.enter_context(tc.tile_pool(name="ost", bufs=3))
        proj_ps = ctx.enter_context(tc.tile_pool(name="proj_ps", bufs=2, space="PSUM"))
        lt_ps = ctx.enter_context(tc.tile_pool(name="lt_ps", bufs=2, space="PSUM"))
        x_ps = ctx.enter_context(tc.tile_pool(name="x_ps", bufs=2, space="PSUM"))

        nc.gpsimd.load_library(library_config.attn)

        # pre-warm the Exp activation table while DMAs stream
        pw_in = const.tile([1, 16], f32, tag="pw_in")
        pw_out = const.tile([1, 16], f32, tag="pw_out")
        nc.vector.memset(pw_in, 0.0)
        nc.scalar.activation(pw_out, pw_in, EXP)
        # warm the PE p-state ramp with throwaway matmuls while DMAs stream
        wu = const.tile([P, P], bf16, tag="wu")
        nc.vector.memset(wu, 0.0)

        # ---- weights + inputs to SBUF.  The sync DMA queue drains in
        # emission order; this order paces the whole schedule.
        wq_s = const.tile([P, KT, HB], bf16, tag="wq")
        wk_s = const.tile([P, KT, HB], bf16, tag="wk")
        wv_s = const.tile([P, KT, HB], bf16, tag="wv")
        wo_s = const.tile([P, HB // P, C], bf16, tag="wo")
        xv_s = const.tile([P, KT, NK], bf16, tag="xv")

        def csl(c):
            return slice(c * QCH, (c + 1) * QCH)

        xq_c = [xin.tile([P, KT, QCH], bf16, tag="xq", name=f"xq{c}") for c in range(NCH)]
        xk_c = [xin.tile([P, KT, QCH], bf16, tag="xk", name=f"xk{c}") for c in range(NCH)]

        # only the mt=0 weight slices gate the first projections
        nc.sync.dma_start(out=xq_c[0], in_=xq_d[0, :, :, :])
        nc.sync.dma_start(out=wq_s[:, :, 0:P], in_=wq_d[:, :, 0:P])
        nc.sync.dma_start(out=wk_s[:, :, 0:P], in_=wk_d[:, :, 0:P])
        nc.sync.dma_start(out=xk_c[0], in_=xk_d[0, :, :, :])
        nc.sync.dma_start(out=wv_s, in_=wv_d[:, :, :])
        nc.sync.dma_start(out=xv_s[:, :, 0 : NK // 2], in_=xv_d[:, :, 0 : NK // 2])
        nc.sync.dma_start(out=xk_c[1], in_=xk_d[1, :, :, :])
        nc.sync.dma_start(out=xv_s[:, :, NK // 2 : NK], in_=xv_d[:, :, NK // 2 : NK])
        nc.sync.dma_start(out=xk_c[2], in_=xk_d[2, :, :, :])
        nc.sync.dma_start(out=xk_c[3], in_=xk_d[3, :, :, :])
        nc.sync.dma_start(out=xq_c[1], in_=xq_d[1, :, :, :])
        nc.sync.dma_start(out=wq_s[:, :, P:HB], in_=wq_d[:, :, P:HB])
        nc.sync.dma_start(out=wk_s[:, :, P:HB], in_=wk_d[:, :, P:HB])
        nc.sync.dma_start(out=xq_c[2], in_=xq_d[2, :, :, :])
        nc.sync.dma_start(out=xq_c[3], in_=xq_d[3, :, :, :])
        nc.sync.dma_start(out=wo_s, in_=wo_d[:, :, :])

        # v with a ones column per head: [128, kt, head, 65]
        v_s = const.tile([P, NKT, HPC, VW], bf16, tag="v")
        nc.vector.memset(v_s[:, :, :, DH : DH + 1], 1.0)

        qT_t = [qk.tile([P, NQ], bf16, tag="qT", name=f"qT{i}") for i in range(3)]
        kT_t = [qk.tile([P, NK], bf16, tag="kT", name=f"kT{i}") for i in range(3)]
        xn_t = [xnp.tile([P, NQ], bf16, tag="xn", name=f"xn{i}") for i in range(3)]

        def proj_chunk(w_s, src_c, dst, mt, c):
            ps = proj_ps.tile([P, QCH], f32, tag="proj")
            for k in range(KT):
                nc.tensor.matmul(
                    ps,
                    w_s[:, k, mt * P : (mt + 1) * P],
                    src_c[c][:, k, :],
                    start=(k == 0),
                    stop=(k == KT - 1),
                )
            nc.vector.tensor_copy(dst[:, csl(c)], ps)

        def v_proj_kt(kt):
            ps = proj_ps.tile([P, QCH], f32, tag="proj")
            psv = ps[:, 0:HB]
            for k in range(KT):
                nc.tensor.matmul(
                    psv,
                    xv_s[:, k, kt * P : (kt + 1) * P],
                    wv_s[:, k, :],
                    start=(k == 0),
                    stop=(k == KT - 1),
                )
            nc.vector.tensor_copy(
                v_s[:, kt, :, 0:DH], psv.rearrange("p (h m) -> p h m", m=DH)
            )

        rows = (slice(0, DH), slice(DH, 2 * DH))
        p_store = {}  # (pair, chunk) -> [hh][ktp] p tiles

        def qk_ktp(p3, c, ktp):
            # logits for key tiles (2*ktp, 2*ktp+1), both heads of the pair
            lts = [
                lt_ps.tile([P, 2 * QCH], f32, tag="lt", name=f"lt{i}") for i in range(2)
            ]
            for u in range(2):
                kt = 2 * ktp + u
                for hh in range(2):
                    nc.tensor.matmul(
                        lts[hh][:, u * QCH : (u + 1) * QCH],
                        kT_t[p3][rows[hh], kt * P : (kt + 1) * P],
                        qT_t[p3][rows[hh], csl(c)],
                        start=True,
                        stop=True,
                    )
            for hh in range(2):
                pt = pp.tile([P, 2 * QCH], bf16, tag="p")
                # attention scale folded into the exp: exp(logit/8)
                nc.scalar.activation(pt, lts[hh], EXP, scale=SCALE)
                p_store[(p3, c)][hh].append(pt)

        def qk_unit(p3, c, ktps=range(NKT // 2)):
            if (p3, c) not in p_store:
                p_store[(p3, c)] = ([], [])
            for ktp in ktps:
                qk_ktp(p3, c, ktp)

        def av_norm(p3, c, hh, ptiles, q0, qw):
            # AV chain + normalize for query columns [q0, q0+qw) of chunk c
            h = 2 * p3 + hh
            xps = x_ps.tile([DH + 1, QCH], f32, tag="x")
            xpsw = xps[:, 0:qw]
            for kt in range(NKT):
                pt = ptiles[hh][kt // 2][:, (kt % 2) * QCH + q0 :][:, 0:qw]
                nc.tensor.matmul(
                    xpsw,
                    v_s[:, kt, h, :],
                    pt,
                    start=(kt == 0),
                    stop=(kt == NKT - 1),
                )
            qsl = slice(c * QCH + q0, c * QCH + q0 + qw)
            # copy x+denom to SBUF (frees the PSUM bank); the custom-DVE
            # recip and gpsimd broadcast mis-handle non-zero partition bases
            # on HW, so DMA the denom row to a partition-0 tile first.
            xsb = small.tile([DH + 1, QCH], f32, tag="xsb", bufs=3)
            nc.vector.tensor_copy(xsb[:, 0:qw], xpsw)
            r0 = small.tile([1, QCH], f32, tag="r0")
            nc.sync.dma_start(out=r0[:, 0:qw], in_=xsb[DH : DH + 1, 0:qw])
            r = small.tile([1, QCH], f32, tag="r")
            nc.vector.reciprocal_approx_fast(r[:, 0:qw], r0[:, 0:qw])
            rb = small.tile([DH, QCH], f32, tag="rb", bufs=2)
            nc.gpsimd.partition_broadcast(rb[:, 0:qw], r[:, 0:qw])
            if hh == 0:
                nc.vector.tensor_mul(xn_t[p3][0:DH, qsl], xsb[0:DH, 0:qw], rb[:, 0:qw])
            else:
                # DVE lanes are partition-locked; route rows 64..127
                # through a bounce tile + SBUF-to-SBUF DMA.
                tmp = small.tile([DH, QCH], bf16, tag="tmp")
                nc.vector.tensor_mul(tmp[:, 0:qw], xsb[0:DH, 0:qw], rb[:, 0:qw])
                nc.sync.dma_start(out=xn_t[p3][DH : 2 * DH, qsl], in_=tmp[:, 0:qw])

        def av_unit(p3, c):
            ptiles = p_store.pop((p3, c))
            for hh in range(2):
                av_norm(p3, c, hh, ptiles, 0, QCH)

        def outproj(c, q0=0, qw=QCH):
            qsl = slice(c * QCH + q0, c * QCH + q0 + qw)
            for mt in range(C // P):
                ps = proj_ps.tile([P, QCH], f32, tag="proj")
                psw = ps[:, 0:qw]
                for k3 in range(HB // P):
                    nc.tensor.matmul(
                        psw,
                        wo_s[:, k3, mt * P : (mt + 1) * P],
                        xn_t[k3][:, qsl],
                        start=(k3 == 0),
                        stop=(k3 == HB // P - 1),
                    )
                o = ost.tile([P, QCH], f32, tag="o")
                nc.vector.tensor_copy(o[:, 0:qw], psw)
                nc.sync.dma_start(
                    out=out_d[mt * P : (mt + 1) * P, qsl], in_=o[:, 0:qw]
                )

        # ---- static schedule.
        # Head: q0/k0 projections chase the chunked DMAs; QK ktp j only
        # needs key chunk j//2, so the exp stream starts ~10us in.
        for _ in range(30):
            wps = proj_ps.tile([P, QCH], f32, tag="proj")
            nc.tensor.matmul(wps[:, 0:P], wu, wu, start=True, stop=True)
        proj_chunk(wq_s, xq_c, qT_t[0], 0, 0)
        proj_chunk(wk_s, xk_c, kT_t[0], 0, 0)
        qk_unit(0, 0, ktps=(0, 1))
        proj_chunk(wk_s, xk_c, kT_t[0], 0, 1)
        qk_unit(0, 0, ktps=(2, 3))
        # v-projection rides inside the u0 QK window (which otherwise waits
        # on the chunked xk DMAs); AV(u0) halves land right where exp(u0)
        # drains so the p-tile pool frees for exp(u1) without a stall
        for kt in range(0, 4):
            v_proj_kt(kt)
        proj_chunk(wk_s, xk_c, kT_t[0], 0, 2)
        qk_unit(0, 0, ktps=(4, 5))
        for kt in range(4, 8):
            v_proj_kt(kt)
        proj_chunk(wk_s, xk_c, kT_t[0], 0, 3)
        qk_unit(0, 0, ktps=(6, 7))
        for kt in range(8, 12):
            v_proj_kt(kt)
        proj_chunk(wq_s, xq_c, qT_t[0], 0, 1)
        qk_unit(0, 1, ktps=(0, 1))
        for kt in range(12, 16):
            v_proj_kt(kt)
        pt_u0 = p_store.pop((0, 0))
        av_norm(0, 0, 0, pt_u0, 0, QCH)
        qk_ktp(0, 1, 2)
        av_norm(0, 0, 1, pt_u0, 0, QCH)
        for j in range(3, 8):
            qk_ktp(0, 1, j)

        units = [(p3, c) for p3 in range(3) for c in range(NCH)]
        # prerequisite projection chunks emitted just before QK(unit i);
        # the q/k chunks for pair p+1 are spread across pair p's units.
        fillers = {
            2: [("q", 0, 2), ("k", 1, 0)],
            3: [("q", 0, 3), ("k", 1, 1), ("k", 1, 2)],
            4: [("k", 1, 3), ("q", 1, 0)],
            5: [("q", 1, 1), ("k", 2, 0)],
            6: [("q", 1, 2), ("k", 2, 1)],
            7: [("q", 1, 3), ("k", 2, 2)],
            8: [("k", 2, 3), ("q", 2, 0)],
            9: [("q", 2, 1)],
            10: [("q", 2, 2)],
            11: [("q", 2, 3)],
        }

        def emit_filler(i):
            for f in fillers.get(i, ()):
                if f[0] == "q":
                    proj_chunk(wq_s, xq_c, qT_t[f[1]], f[1], f[2])
                else:
                    proj_chunk(wk_s, xk_c, kT_t[f[1]], f[1], f[2])

        pending_outproj = None
        for i, (p3, c) in enumerate(units):
            if i == 0:
                continue  # handled in the prologue above
            if i + 1 < len(units):
                emit_filler(i + 1)
                qk_unit(*units[i + 1])
            if pending_outproj is not None:
                outproj(pending_outproj)
                pending_outproj = None
            if i < len(units) - 1:
                av_unit(p3, c)
            else:
                # last unit: process in half-chunks so the final
                # output-projection overlaps the second half's AV/normalize
                ptiles = p_store.pop((p3, c))
                HW_ = QCH // 2
                for hh in range(2):
                    av_norm(p3, c, hh, ptiles, 0, HW_)
                outproj(c, 0, HW_)
                for hh in range(2):
                    av_norm(p3, c, hh, ptiles, HW_, HW_)
                outproj(c, HW_, HW_)
            if p3 == 2:
                pending_outproj = c

    nc.finalize()
    return nc


def _get_prog():
    global _prog
    if _prog is None:
        _prog = _build()
    return _prog


def _shard_inputs(query, key, value, Wq, Wk, Wv, Wo):
    from ml_dtypes import bfloat16

    def qtile(x):  # [NQ, C] -> [NCH, P, KT, QCH], per-partition contiguous
        return np.ascontiguousarray(
            x.reshape(NCH, QCH, KT, P).transpose(0, 3, 2, 1)
        ).astype(bfloat16)

    def vtile(x):  # [NK, C] -> [P, KT, NK]
        return np.ascontiguousarray(x.T.reshape(KT, P, NK).transpose(1, 0, 2)).astype(
            bfloat16
        )

    def wtile(w):  # [C, HB] -> [P, KT, HB]
        return np.ascontiguousarray(w.reshape(KT, P, HB).transpose(1, 0, 2)).astype(
            bfloat16
        )

    in_maps = []
    for core in range(8):
        b, hg = core // 2, core % 2
        sl = slice(hg * HB, (hg + 1) * HB)
        wo_t = Wo[:, sl].T  # [HB, C]
        in_maps.append(
            {
                "xq": qtile(np.asarray(query[b])),
                "xk": qtile(np.asarray(key[b])),
                "xv": vtile(np.asarray(value[b])),
                "wq": wtile(np.ascontiguousarray(Wq[sl, :].T)),
                "wk": wtile(np.ascontiguousarray(Wk[sl, :].T)),
                "wv": wtile(np.ascontiguousarray(Wv[sl, :].T)),
                "wo": np.ascontiguousarray(
                    wo_t.reshape(HB // P, P, C).transpose(1, 0, 2)
                ).astype(bfloat16),
            }
        )
    return in_maps


def kernel(query, key, value, Wq, Wk, Wv, Wo, bo):
    query, key, value = np.asarray(query), np.asarray(key), np.asarray(value)
    Wq, Wk, Wv, Wo = np.asarray(Wq), np.asarray(Wk), np.asarray(Wv), np.asarray(Wo)
    bo = np.asarray(bo).astype(np.float32)

    from concourse.bass_utils import run_bass_kernel_spmd

    nc = _get_prog()
    in_maps = _shard_inputs(query, key, value, Wq, Wk, Wv, Wo)
    res = run_bass_kernel_spmd(nc, in_maps, list(range(8))).results

    out = np.empty((B, NQ, C), np.float32)
    for b in range(B):
        acc = res[2 * b]["out"].astype(np.float32) + res[2 * b + 1]["out"].astype(
            np.float32
        )
        out[b] = acc.T + bo[None, :]
    return out
